# revision 20
# baseline (speedup 1.0000x reference)
"""BiAttention (binary attention transformer block) Trainium2 kernel.

Forward-pass reduction of the reference:
  - softmax cancels:  stop_gradient(binq - soft) + soft == binq  (forward)
  - sign() is invariant to the positive per-row qkv weight scale
So per batch element (one per NeuronCore, 8 cores data-parallel):
  bq,bk,bv = sign(x @ sign(Wqkv).T)   split into heads
  A        = (bq @ bk.T > 0)          in {0,1}
  oo       = A @ bv                   exact small integers
  out      = (oo @ sign(Wproj).T) * mean(|Wproj|,axis=1) + b_proj

Host-side prep (layout only, no matmul FLOPs): x is transposed and split
into fp16 hi + fp8e4m3 lo*2^9 (hi pass at full PE rate, lo pass at fp8
DoubleRow double rate against weight signs scaled 2^-9; end-to-end rel
err ~0.007 vs the 2e-2 gate), weight signs are precomputed as fp8 +-1
(mixed fp8 x fp16 matmuls run at full speed), the proj scale is folded
into the proj weight signs (fp16), and all tensors arrive pre-transposed
so the device spends zero PE cycles on transposes.

Device layout notes:
  - qkv q/k computed transposed [o, n] directly (lhsT=ws chunk, rhs=xT)
  - v-part computed natural [m, hd] (lhsT=xT chunk, rhs=ws v columns)
  - scores per head-pair as maskT [m, n] fp8 {0,1}; the two heads' MMs
    alternate 64-row tile groups (concurrent when the psum rotation
    allows) and their evacs alternate scalar/vector engines
  - A@V via fp8 DoubleRow (K=256/MM); odd head evacs to a temp and
    partition-shifts into ooT with a small SBUF->SBUF DMA
"""

import numpy as np

import concourse.bacc as bacc
import concourse.mybir as mybir
import concourse.tile as tile

FP32 = mybir.dt.float32
FP16 = mybir.dt.float16
FP8 = mybir.dt.float8e4
AF = mybir.ActivationFunctionType
ALU = mybir.AluOpType
DR = mybir.MatmulPerfMode.DoubleRow

B, N, C = 8, 1024, 768
H, D = 12, 64
C3 = 3 * C  # 2304
NK = C // 128  # 6 contraction chunks
NM = N // 128  # 8 token chunks


def build_nc(repeat=1):
    nc = bacc.Bacc("TRN2", target_bir_lowering=False, debug=True)

    # host-prepped inputs (see kernel() below)
    xhi_d = nc.dram_tensor("xt_hi", [C, N], FP16, kind="ExternalInput")
    # lo residual (x - fp16(x)) * 2^9 in fp8e4m3; matching qkv weight signs
    # scaled by 2^-9 so the product accumulates at natural scale
    xlo_d = nc.dram_tensor("xt_lo8", [C, N], FP8, kind="ExternalInput")
    # wsT columns reordered: [ v (768) | hp0: q(128) k(128) | hp1: ... ]
    # ws1: signs as fp8 ±1 (hi pass, mixed fp8 x fp16 matmul)
    # ws9: signs * 2^-9 as fp8, DoubleRow pair layout (lo pass)
    ws1_d = nc.dram_tensor("ws1", [C, C3], FP8, kind="ExternalInput")
    ws9_d = nc.dram_tensor("ws9", [C, C3], FP8, kind="ExternalInput")
    w2t_d = nc.dram_tensor("w2t", [C, C], FP16, kind="ExternalInput")  # scaled
    bias_d = nc.dram_tensor("bias", [1, C], FP32, kind="ExternalInput")
    out_d = nc.dram_tensor("out", [N, C], FP32, kind="ExternalOutput")

    xhi_v = xhi_d[:].rearrange("(c p) n -> p c n", p=128)  # [128, 6, 1024]
    # fp8 lo operands in DoubleRow pair layout: [p, kk, jj, cols],
    # contraction chunk k = 2*kk + jj
    xlo_v = xlo_d[:].rearrange("(a b p) n -> p a b n", p=128, b=2)  # [128,3,2,1024]
    ws1_v = ws1_d[:].rearrange("(c p) f -> p c f", p=128)  # [128, 6, 2304]
    ws9_v = ws9_d[:].rearrange("(a b p) f -> p a b f", p=128, b=2)  # [128,3,2,2304]
    w2t_v = w2t_d[:].rearrange("(c p) f -> p c f", p=128)  # [128, 6, 768]
    out_v = out_d[:].rearrange("(m p) f -> p m f", p=128)  # [128, 8, 768]

    with tile.TileContext(nc) as tc:
        for _rep in range(repeat):
            _emit_body(
                nc, tc, _rep, xhi_v, xlo_v, ws1_v, ws9_v, w2t_v, bias_d, out_v
            )

    nc.compile()
    return nc


def _emit_body(nc, tc, rep, xhi_v, xlo_v, ws1_v, ws9_v, w2t_v, bias_d, out_v):
    _p = f"r{rep}_"
    with (
        tc.tile_pool(name=_p + "persist", bufs=1) as pp,
        tc.tile_pool(name=_p + "qk", bufs=5) as qkp,
        tc.tile_pool(name=_p + "at", bufs=4) as atp,
        tc.tile_pool(name=_p + "outstage", bufs=3) as op,
    ):
        # ---- persistent SBUF ----
        xT_hi = pp.tile([128, NK, N], FP16, tag="xT_hi")  # [c%128, c//128, n]
        xT_lo8 = pp.tile([128, 3, 2, N], FP8, tag="xT_lo8")
        ws1 = pp.tile([128, NK, C3], FP8, tag="ws1")
        ws9 = pp.tile([128, 3, 2, C3], FP8, tag="ws9")
        w2T = pp.tile([128, NK, C], FP16, tag="w2T")
        v_nat = pp.tile([128, NM, C], FP8, tag="v_nat")  # v, ±0.5, [m%128, m//128, hd]
        ooT = pp.tile([128, NK, N], FP16, tag="ooT")  # attn out transposed
        bias_row = pp.tile([1, C], FP32, tag="bias_row")
        bias_rep = pp.tile([128, C], FP32, tag="bias_rep")
        sigb = pp.tile([128, 1], FP32, tag="sigb")

        nc.gpsimd.memset(sigb[:], -32.0)
        nc.sync.dma_start(bias_row[:], bias_d[:])
        nc.gpsimd.partition_broadcast(bias_rep[:], bias_row[:])

        # ---- input DMAs, interleaved so the first v-part chains can start
        # as soon as their k-chunk operands land ----
        for cc in range(NK):
            nc.sync.dma_start(xT_hi[:, cc, :], xhi_v[:, cc, :])
            nc.sync.dma_start(ws1[:, cc, 0:768], ws1_v[:, cc, 0:768])
        for kk in range(3):
            nc.sync.dma_start(xT_lo8[:, kk, :, :], xlo_v[:, kk, :, :])
            nc.sync.dma_start(ws9[:, kk, :, 0:768], ws9_v[:, kk, :, 0:768])
        for hp in range(6):
            o0 = 768 + hp * 256
            nc.sync.dma_start(ws1[:, :, o0 : o0 + 256], ws1_v[:, :, o0 : o0 + 256])
            nc.sync.dma_start(ws9[:, :, :, o0 : o0 + 256], ws9_v[:, :, :, o0 : o0 + 256])
        nc.sync.dma_start(w2T[:], w2t_v[:])

        # ---- v-part: natural orientation [m, hd], fp8 ±0.5 ----
        vp_cm = tc.tile_pool(name=_p + "ps_v", bufs=2, space="PSUM")
        ps_v = vp_cm.__enter__()
        for m in range(NM):
            for half in range(2):
                vp = ps_v.tile([128, 384], FP32, tag="v_ps", name=f"vps{m}_{half}")
                for k in range(NK):
                    nc.tensor.matmul(
                        vp[:],
                        lhsT=xT_hi[:, k, m * 128 : (m + 1) * 128],
                        rhs=ws1[:, k, half * 384 : (half + 1) * 384],
                        start=(k == 0),
                        stop=False,
                    )
                for kk in range(3):
                    nc.tensor.matmul(
                        vp[:],
                        lhsT=xT_lo8[:, kk, :, m * 128 : (m + 1) * 128],
                        rhs=ws9[:, kk, :, half * 384 : (half + 1) * 384],
                        perf_mode=DR,
                        start=False,
                        stop=(kk == 2),
                    )
                nc.vector.tensor_scalar(
                    v_nat[:, m, half * 384 : (half + 1) * 384],
                    vp[:],
                    0.0,
                    0.5,
                    ALU.is_ge,
                    ALU.subtract,
                )
        vp_cm.__exit__(None, None, None)

        # ---- per head-pair: q/k qkv chunks, scores, binarize, A@V ----
        hp_psum_cms = [
            tc.tile_pool(name=_p + "ps_qk", bufs=2, space="PSUM"),
            tc.tile_pool(name=_p + "ps_s", bufs=2, space="PSUM"),
            tc.tile_pool(name=_p + "ps_oo", bufs=2, space="PSUM"),
        ]
        ps_qk, ps_s, ps_oo = [cm.__enter__() for cm in hp_psum_cms]
        bin_idx = 0
        qkTs = {}

        def emit_qk(hp):
            qkT = {}
            for ri, role in enumerate(("q", "k")):
                oc0 = 768 + hp * 256 + ri * 128
                t = qkp.tile([128, N], FP8, tag="qkT", name=f"qkT_{role}{hp}")
                qkT[role] = t
                for ncol in range(2):
                    qp = ps_qk.tile([128, 512], FP32, tag="qk_ps")
                    for k in range(NK):
                        nc.tensor.matmul(
                            qp[:],
                            lhsT=ws1[:, k, oc0 : oc0 + 128],
                            rhs=xT_hi[:, k, ncol * 512 : (ncol + 1) * 512],
                            start=(k == 0),
                            stop=False,
                        )
                    for kk in range(3):
                        nc.tensor.matmul(
                            qp[:],
                            lhsT=ws9[:, kk, :, oc0 : oc0 + 128],
                            rhs=xT_lo8[:, kk, :, ncol * 512 : (ncol + 1) * 512],
                            perf_mode=DR,
                            start=False,
                            stop=(kk == 2),
                        )
                    nc.scalar.activation(
                        t[:, ncol * 512 : (ncol + 1) * 512], qp[:], AF.Sign
                    )
            qkTs[hp] = qkT

        emit_qk(0)
        for hp in range(6):
            qkT = qkTs.pop(hp)
            at = {}
            for h01 in range(2):
                at[h01] = atp.tile([128, NM, N], FP8, tag="at", name=f"at{hp}_{h01}")
            for m in range(NM):
                # emit the two heads' score MMs interleaved so consecutive
                # MMs sit on different 64-row groups and run concurrently
                sp_ps = [
                    ps_s.tile([128, N], FP32, tag="s_ps", name=f"sps{hp}_{m}_{h01}")
                    for h01 in range(2)
                ]
                for ncol in range(2):
                    for h01 in range(2):
                        ph = 64 * h01
                        nc.tensor.matmul(
                            sp_ps[h01][:, ncol * 512 : (ncol + 1) * 512],
                            lhsT=qkT["k"][ph : ph + 64, m * 128 : (m + 1) * 128],
                            rhs=qkT["q"][ph : ph + 64, ncol * 512 : (ncol + 1) * 512],
                            tile_position=(ph, 0),
                        )
                for h01 in range(2):
                    dst = at[h01][:, m, :]
                    # the two heads' evacs must go to different engines so
                    # they drain concurrently (they gate the tile rotation)
                    if h01 == 0:
                        nc.scalar.activation(
                            dst, sp_ps[h01][:], AF.Sigmoid, bias=sigb[:], scale=32.0
                        )
                    else:
                        nc.vector.tensor_scalar(
                            dst, sp_ps[h01][:], 0.0, None, ALU.is_gt
                        )
                    bin_idx += 1

            if hp + 1 < 6:
                emit_qk(hp + 1)

            for h01 in range(2):
                h = 2 * hp + h01
                if h01 == 1:
                    oo_tmp = op.tile([64, N], FP16, tag="oo_tmp", name=f"oo_tmp{hp}")
                for ncol in range(2):
                    oo_ps = ps_oo.tile(
                        [64, 512], FP32, tag="oo_ps", name=f"oo_ps{hp}_{h01}_{ncol}"
                    )
                    for j in range(4):
                        nc.tensor.matmul(
                            oo_ps[:],
                            lhsT=v_nat[:, 2 * j : 2 * j + 2, h * 64 : (h + 1) * 64],
                            rhs=at[h01][:, 2 * j : 2 * j + 2, ncol * 512 : (ncol + 1) * 512],
                            perf_mode=DR,
                            start=(j == 0),
                            stop=(j == 3),
                        )
                    # v was ±0.5 -> x2 recovers exact integer attention out;
                    # odd head's lanes must land on partitions 64-127: evac to
                    # a temp then partition-shift with a small SBUF->SBUF DMA
                    dsth = ooT[0:64, hp, :] if h01 == 0 else oo_tmp[:]
                    csl = dsth[:, ncol * 512 : (ncol + 1) * 512]
                    if ncol == 0:
                        nc.scalar.activation(csl, oo_ps[:], AF.Copy, scale=2.0)
                    else:
                        nc.vector.tensor_scalar(csl, oo_ps[:], 2.0, None, ALU.mult)
                if h01 == 1:
                    nc.sync.dma_start(ooT[64:128, hp, :], oo_tmp[:])
        for cm in reversed(hp_psum_cms):
            cm.__exit__(None, None, None)

        # ---- projection (scale pre-folded into w2T on host) ----
        with tc.tile_pool(name=_p + "ps_proj", bufs=2, space="PSUM") as ps_p:
            for m in range(NM):
                ot = op.tile([128, C], FP32, tag="out_stage")
                for n0, nw in ((0, 512), (512, 256)):
                    pps = ps_p.tile([128, nw], FP32, tag=f"p_ps{n0}")
                    for k in range(NK):
                        nc.tensor.matmul(
                            pps[:],
                            lhsT=ooT[:, k, m * 128 : (m + 1) * 128],
                            rhs=w2T[:, k, n0 : n0 + nw],
                            start=(k == 0),
                            stop=(k == NK - 1),
                        )
                    nc.vector.scalar_tensor_tensor(
                        ot[:, n0 : n0 + nw],
                        pps[:],
                        1.0,
                        bias_rep[:, n0 : n0 + nw],
                        ALU.bypass,
                        ALU.add,
                    )
                nc.sync.dma_start(out_v[:, m, :], ot[:])


_CACHE = {}


def _get_exec():
    """Build (once) and cache a jitted SPMD executable for the 8-core kernel."""
    if "exec" in _CACHE:
        return _CACHE["exec"]
    import jax
    import concourse.mybir as _mybir
    from jax.sharding import Mesh, PartitionSpec
    from jax.experimental.shard_map import shard_map
    from concourse.bass2jax import _bass_exec_p, install_neuronx_cc_hook

    nc = build_nc()
    install_neuronx_cc_hook()

    in_names, out_names, out_avals = [], [], []
    for alloc in nc.m.functions[0].allocations:
        if not isinstance(alloc, _mybir.MemoryLocationSet):
            continue
        name = alloc.memorylocations[0].name
        if alloc.kind == "ExternalInput":
            if name not in ("dbg_addr", "partition_id"):
                in_names.append(name)
        elif alloc.kind == "ExternalOutput":
            out_names.append(name)
            out_avals.append(
                jax.core.ShapedArray(tuple(alloc.tensor_shape), _mybir.dt.np(alloc.dtype))
            )
    if nc.dbg_addr is not None:
        in_names.append(nc.dbg_addr.name)
    n_params = len(in_names)
    n_outs = len(out_names)
    partition_name = nc.partition_id_tensor.name if nc.partition_id_tensor else None
    all_in_names = tuple(
        in_names + out_names + ([partition_name] if partition_name else [])
    )
    donate = tuple(range(n_params, n_params + n_outs))

    def _body(*args):
        operands = list(args)
        if partition_name is not None:
            from concourse.bass2jax import partition_id_tensor

            operands.append(partition_id_tensor())
        outs = _bass_exec_p.bind(
            *operands,
            out_avals=tuple(out_avals),
            in_names=all_in_names,
            out_names=tuple(out_names),
            lowering_input_output_aliases=(),
            sim_require_finite=True,
            sim_require_nnan=True,
            nc=nc,
        )
        return tuple(outs)

    devices = jax.devices()[:B]
    mesh = Mesh(np.array(devices), ("core",))
    in_specs = (PartitionSpec("core"),) * (n_params + n_outs)
    out_specs = (PartitionSpec("core"),) * n_outs
    sharded = jax.jit(
        shard_map(_body, mesh=mesh, in_specs=in_specs, out_specs=out_specs, check_rep=False),
        donate_argnums=donate,
        keep_unused=True,
    )
    _CACHE["exec"] = (sharded, in_names, out_names, out_avals, mesh)
    return _CACHE["exec"]


def _host_prep(x, w_qkv, w_proj, b_proj):
    """Layout-only host prep: transposes, fp16/fp8 hi/lo split, weight signs."""
    import ml_dtypes

    FP8NP = ml_dtypes.float8_e4m3

    x = np.asarray(x, np.float32)
    w_qkv = np.asarray(w_qkv, np.float32)
    w_proj = np.asarray(w_proj, np.float32)
    b_proj = np.asarray(b_proj, np.float32).reshape(1, C)

    # qkv weight signs, transposed, with columns reordered:
    # [ v (768) | hp0: q(128) k(128) | hp1: q k | ... ]
    ws = np.where(w_qkv >= 0, np.float16(1.0), np.float16(-1.0))  # [2304, 768]
    q_s, k_s, v_s = ws[0:C], ws[C : 2 * C], ws[2 * C :]
    cols = [v_s]
    for hp in range(6):
        cols.append(q_s[hp * 128 : (hp + 1) * 128])
        cols.append(k_s[hp * 128 : (hp + 1) * 128])
    wst = np.concatenate(cols, axis=0).T.astype(np.float32)  # [768, 2304]
    # fp8 signs: ±1 for the hi pass, ±2^-9 (exactly representable) for the
    # lo pass -- both exact in fp8e4m3
    ws1 = np.ascontiguousarray(wst.astype(FP8NP))
    ws9 = np.ascontiguousarray((wst * 2.0**-9).astype(FP8NP))

    # proj: fold per-row scale into the sign matrix (fp16 rounding of the
    # scale is ~2^-12 relative -- far inside tolerance)
    sc2 = np.abs(w_proj).mean(axis=1, dtype=np.float64).astype(np.float32)
    w2 = np.where(w_proj >= 0, 1.0, -1.0).astype(np.float32) * sc2[:, None]
    w2t = np.ascontiguousarray(w2.T.astype(np.float16))  # [768, 768]

    # x per batch: transpose, fp16 hi + scaled fp8 lo split
    xt = np.ascontiguousarray(x.transpose(0, 2, 1))  # [B, 768, 1024]
    xt_hi = xt.astype(np.float16)
    xt_lo8 = ((xt - xt_hi.astype(np.float32)) * 512.0).astype(FP8NP)
    return xt_hi, xt_lo8, ws1, ws9, w2t, b_proj


def _concat_inputs(x, w_qkv, w_proj, b_proj):
    """Per-core inputs concatenated along axis 0 (shard_map convention)."""
    xt_hi, xt_lo8, ws1, ws9, w2t, bias = _host_prep(x, w_qkv, w_proj, b_proj)
    per_core = {
        "xt_hi": [np.ascontiguousarray(xt_hi[b]) for b in range(B)],
        "xt_lo8": [np.ascontiguousarray(xt_lo8[b]) for b in range(B)],
        "ws1": [ws1] * B,
        "ws9": [ws9] * B,
        "w2t": [w2t] * B,
        "bias": [bias] * B,
        "dbg_addr": [np.zeros((1, 2), np.uint32)] * B,
    }
    return per_core


def _zero_outs(out_names, out_avals):
    return [
        np.zeros((B * a.shape[0], *a.shape[1:]), a.dtype) for a in out_avals
    ]


def kernel(x, w_qkv, w_proj, b_proj):
    sharded, in_names, out_names, out_avals, mesh = _get_exec()
    per_core = _concat_inputs(x, w_qkv, w_proj, b_proj)
    concat_in = [np.concatenate(per_core[name], axis=0) for name in in_names]
    out_arrs = sharded(*concat_in, *_zero_outs(out_names, out_avals))
    i = out_names.index("out")
    a = out_avals[i]
    return np.asarray(out_arrs[i]).reshape(B, *a.shape)
